# revision 10
# baseline (speedup 1.0000x reference)
"""Trainium2 Bass kernel for nn_BiLSTM_CRF (CRF negative log-likelihood loss).

Problem: loss = mean_b( logZ_b - gold_b ) for a linear-chain CRF with
B=512 sequences, T=512 steps, K=128 tags (START=126, STOP=127).

Algorithm (per core, data-parallel over batch, 64 sequences/core):
  The log-semiring forward scan is computed in the exp domain so each step
  is one 128x128x64 TensorE matmul with a *fixed* stationary weight
  W = exp(transitions^T - c), where c is a constant per-step shift that
  keeps exp-domain magnitudes in fp32/bf16 range (the per-step log-growth
  of the partition function is ~c; measured drift stays within +-7 log
  units over all 512 steps, far inside bf16/fp32 exponent range):

      A_0 = onehot(START);  A_{t+1} = exp(feats_t) ⊙ (W @ A_t)
      logZ = log(colsum(A_T ⊙ exp(T[STOP,:]))) + T*c

  Gold-path score splits into:
    - emit  = sum_t feats[b,t,tags[b,t]]      -> on device (touches feats):
      one fused DVE/GPSIMD scalar_tensor_tensor per 128-row block:
      (iota_k == tag_p) * feats_nat with accum_out giving the free-dim sum.
    - trans = sum_t T[tag_t,tag_{t-1}] (+STOP) -> on host (64KB table gather).

The final mean over batch is a host-side sum of the 8 per-core partials.
"""

import numpy as np
import ml_dtypes

import concourse.bass as bass
from concourse import bacc
import concourse.mybir as mybir
import concourse.tile as tile
from concourse.alu_op_type import AluOpType

B, T, K = 512, 512, 128
NCORES = 8
BPC = B // NCORES  # 64 sequences per core
START, STOP = K - 2, K - 1

# Constant per-step shift: E[logZ]/T measured on the problem's data
# distribution (randn feats/transitions). Any value within ~0.1 of the true
# mean growth keeps the scan in range; measured drift with this value is
# [-6.7, +5.9] log units.
C_SHIFT = 5.826096

TSEG = 64               # scan timesteps per exp() segment
NSEG = T // TSEG
NBLK = BPC * T // 128   # 256 natural-layout 128-row blocks for emit score
BLK_GRP = 8             # natural blocks DMA'd together
F32 = mybir.dt.float32
BF16 = mybir.dt.bfloat16

_NC_CACHE = {}


def build_kernel(emit_on_gpsimd=False):
    key = ("nc", emit_on_gpsimd)
    if key in _NC_CACHE:
        return _NC_CACHE[key]
    nc = bacc.Bacc(None, target_bir_lowering=False)
    AF = mybir.ActivationFunctionType

    feats_d = nc.dram_tensor("feats", [BPC * T, K], BF16, kind="ExternalInput")
    tags_d = nc.dram_tensor("tagsT", [128, NBLK], BF16, kind="ExternalInput")
    trans_d = nc.dram_tensor("transT", [K, K], F32, kind="ExternalInput")
    out_d = nc.dram_tensor("out", [1, BPC], F32, kind="ExternalOutput")
    emit_d = nc.dram_tensor("emitcols", [128, NBLK], F32, kind="ExternalOutput")

    with tile.TileContext(nc) as tc:
        with (
            tc.tile_pool(name="const", bufs=1) as cpool,
            tc.tile_pool(name="big", bufs=1) as bigpool,
            tc.tile_pool(name="seg", bufs=2) as segpool,
            tc.tile_pool(name="nat", bufs=4) as natpool,
            tc.tile_pool(name="apool", bufs=3) as apool,
            tc.tile_pool(name="scr", bufs=8) as scrpool,
            tc.tile_pool(name="psum", bufs=4, space="PSUM") as psum_pool,
            tc.tile_pool(name="psumf", bufs=1, space="PSUM") as psum_fin,
        ):
            # ---- constants ----
            # transT input is transitions^T - c (host pre-shifted), so W and
            # stopcol are both exp() of it; logZ = logS + (T+1)*c on host.
            transT_s = cpool.tile([K, K], F32)
            nc.sync.dma_start(out=transT_s, in_=trans_d[:])
            W = cpool.tile([K, K], BF16)  # [prev, next] = exp(T^T - c)
            nc.scalar.activation(W, transT_s, AF.Exp)
            stopcol = cpool.tile([K, 1], F32)  # exp(T[STOP, k] - c) per partition k
            nc.scalar.activation(stopcol, transT_s[:, STOP : STOP + 1], AF.Exp)
            ones_b = cpool.tile([K, 1], BF16)
            nc.vector.memset(ones_b, 1.0)
            iota_k = cpool.tile([K, K], BF16)  # iota_k[p, j] = j
            nc.gpsimd.iota(
                iota_k,
                pattern=[[1, K]],
                base=0,
                channel_multiplier=0,
                allow_small_or_imprecise_dtypes=True,
            )
            tags_s = cpool.tile([128, NBLK], BF16)
            nc.sync.dma_start(out=tags_s, in_=tags_d[:])
            emit_cols = bigpool.tile([128, NBLK], F32)

            # ---- resident transposed feats [k, b*T + t] ----
            featsT = bigpool.tile([K, BPC * T], BF16)
            for b in range(BPC):
                nc.sync.dma_start(
                    out=featsT[:, b * T : (b + 1) * T],
                    in_=feats_d[b * T : (b + 1) * T, :],
                    transpose=True,
                )
            featsT_tb = featsT.rearrange("p (b t) -> p t b", t=T)  # [K, T, BPC]

            # ---- A0 = onehot(START): fill 1.0 where partition == START ----
            A = apool.tile([K, BPC], BF16)
            nc.gpsimd.memset(A, 0.0)
            nc.gpsimd.affine_select(
                out=A,
                in_=A,
                compare_op=AluOpType.not_equal,
                fill=1.0,
                base=-START,
                channel_multiplier=1,
                pattern=[[0, BPC]],
            )

            # ---- emit score: natural-layout blocks, fused compare*mul+rowsum ----
            emit_eng = nc.gpsimd if emit_on_gpsimd else nc.vector
            for g in range(NBLK // BLK_GRP):
                nat = natpool.tile([128, BLK_GRP, K], BF16)
                nc.sync.dma_start(
                    out=nat,
                    in_=feats_d[g * BLK_GRP * 128 : (g + 1) * BLK_GRP * 128, :].rearrange(
                        "(j p) k -> p j k", j=BLK_GRP
                    ),
                )
                for j in range(BLK_GRP):
                    col = g * BLK_GRP + j
                    scr = scrpool.tile([128, K], BF16)
                    emit_eng.scalar_tensor_tensor(
                        out=scr,
                        in0=iota_k,
                        scalar=tags_s[:, col : col + 1],
                        in1=nat[:, j, :],
                        op0=AluOpType.is_equal,
                        op1=AluOpType.mult,
                        accum_out=emit_cols[:, col : col + 1],
                    )

            # ---- the scan ----
            for s in range(NSEG):
                expF = segpool.tile([K, TSEG * BPC], F32)
                nc.scalar.activation(
                    expF.rearrange("p (t b) -> p t b", b=BPC),
                    featsT_tb[:, s * TSEG : (s + 1) * TSEG, :],
                    AF.Exp,
                )
                for ti in range(TSEG):
                    psum_M = psum_pool.tile([K, BPC], F32)
                    nc.tensor.matmul(psum_M, W, A, start=True, stop=True)
                    A_new = apool.tile([K, BPC], BF16)
                    nc.vector.tensor_mul(
                        A_new, psum_M, expF[:, ti * BPC : (ti + 1) * BPC]
                    )
                    A = A_new

            # ---- finalize: logS = log(colsum(A ⊙ stopcol)) ----
            Afin = apool.tile([K, BPC], BF16)
            nc.vector.tensor_scalar_mul(Afin, A, stopcol)
            psum_S = psum_fin.tile([1, BPC], F32)
            nc.tensor.matmul(psum_S, ones_b, Afin, start=True, stop=True)
            logS = cpool.tile([1, BPC], F32)
            nc.scalar.activation(logS, psum_S, AF.Ln)
            nc.sync.dma_start(out=out_d[:], in_=logS)
            nc.sync.dma_start(out=emit_d[:], in_=emit_cols)

    nc.compile()
    nc.finalize()
    _NC_CACHE[key] = nc
    return nc


def prep_inputs(feats, tags, transitions):
    """Host-side marshalling: slice per core, cast, transpose."""
    feats_bf = np.asarray(feats, dtype=np.float32).astype(ml_dtypes.bfloat16)
    tags64 = np.asarray(tags).astype(np.int64)
    transT = np.ascontiguousarray(
        np.asarray(transitions, dtype=np.float32).T - np.float32(C_SHIFT)
    )
    in_maps = []
    for c in range(NCORES):
        fb = np.ascontiguousarray(feats_bf[c * BPC : (c + 1) * BPC].reshape(BPC * T, K))
        tg = np.ascontiguousarray(
            tags64[c * BPC : (c + 1) * BPC]
            .reshape(NBLK, 128)
            .T.astype(ml_dtypes.bfloat16)
        )
        in_maps.append({"feats": fb, "tagsT": tg, "transT": transT})
    return in_maps, tags64


def combine_outputs(results, tags64, transitions):
    """Host-side: per-core logS/emit partials + trans gold score -> loss."""
    Trf = np.asarray(transitions, dtype=np.float64)
    ext = np.concatenate([np.full((B, 1), START, np.int64), tags64], axis=1)
    trans_gold = Trf[ext[:, 1:], ext[:, :-1]].sum(axis=1) + Trf[STOP, ext[:, -1]]
    total = 0.0
    for c in range(NCORES):
        logS = results[c]["out"][0].astype(np.float64)  # [BPC]
        ecols = results[c]["emitcols"].astype(np.float64)  # [128, NBLK]
        emit_b = ecols.sum(axis=0).reshape(BPC, 4).sum(axis=1)
        logZ = logS + (T + 1) * C_SHIFT
        total += float(np.sum(logZ - emit_b - trans_gold[c * BPC : (c + 1) * BPC]))
    return np.asarray(total / B, dtype=np.float32)


def kernel(feats, tags, transitions):
    from concourse.bass_utils import run_bass_kernel_spmd

    nc = build_kernel()
    in_maps, tags64 = prep_inputs(feats, tags, transitions)
    res = run_bass_kernel_spmd(nc, in_maps, list(range(NCORES)))
    return combine_outputs(res.results, tags64, transitions)


if __name__ == "__main__":
    nc = build_kernel()
    print("kernel built and compiled OK")


# revision 11
# speedup vs baseline: 1.1772x; 1.1772x over previous
"""Trainium2 Bass kernel for nn_BiLSTM_CRF (CRF negative log-likelihood loss).

Problem: loss = mean_b( logZ_b - gold_b ) for a linear-chain CRF with
B=512 sequences, T=512 steps, K=128 tags (START=126, STOP=127).

Algorithm (per core, data-parallel over batch, 64 sequences/core):
  The log-semiring forward scan is computed in the exp domain so each step
  is one 128x128x64 TensorE matmul with a *fixed* stationary weight
  W = exp(transitions^T - c), where c is a constant per-step shift that
  keeps exp-domain magnitudes in fp32/bf16 range (the per-step log-growth
  of the partition function is ~c; measured drift stays within +-7 log
  units over all 512 steps, far inside bf16/fp32 exponent range):

      A_0 = onehot(START);  A_{t+1} = exp(feats_t) ⊙ (W @ A_t)
      logZ = log(colsum(A_T ⊙ exp(T[STOP,:] - c))) + (T+1)*c

  Gold-path score splits into:
    - emit  = sum_t feats[b,t,tags[b,t]]      -> on device (touches feats):
      one fused DVE scalar_tensor_tensor per 128-row block:
      (iota_k == tag_p) * feats_nat with accum_out giving the free-dim sum.
      Emit ops are interleaved 1:2 with scan steps so they fill the DVE
      gaps between the scan's PSUM-evacuation multiplies.
    - trans = sum_t T[tag_t,tag_{t-1}] (+STOP) -> on host (64KB table gather).

feats is shipped twice in bf16 (transposed [K, t-major(T,B)] for the scan's
matmul/exp pipeline, natural [B*T, K] for emit) -- 16MB/core of DMA, fully
hidden under the ~512-step scan chain.

The final mean over batch is a host-side sum of the 8 per-core partials.
"""

import numpy as np
import ml_dtypes

import concourse.bass as bass
from concourse import bacc
import concourse.mybir as mybir
import concourse.tile as tile
from concourse.alu_op_type import AluOpType

B, T, K = 512, 512, 128
NCORES = 8
BPC = B // NCORES  # 64 sequences per core
START, STOP = K - 2, K - 1

# Constant per-step shift: E[logZ]/T measured on the problem's data
# distribution (randn feats/transitions). Any value within ~0.1 of the true
# mean growth keeps the scan in range; measured drift with this value is
# [-6.7, +5.9] log units.
C_SHIFT = 5.826096

TSEG = 32               # scan timesteps per exp() segment
NSEG = T // TSEG
NBLK = BPC * T // 128   # 256 natural-layout 128-row blocks for emit score
BLK_GRP = 8             # natural blocks DMA'd together
F32 = mybir.dt.float32
BF16 = mybir.dt.bfloat16

_NC_CACHE = {}


def build_kernel():
    key = "nc"
    if key in _NC_CACHE:
        return _NC_CACHE[key]
    nc = bacc.Bacc(None, target_bir_lowering=False)
    AF = mybir.ActivationFunctionType

    featsT_d = nc.dram_tensor("featsT", [K, T * BPC], BF16, kind="ExternalInput")
    featsN_d = nc.dram_tensor("featsN", [BPC * T, K], BF16, kind="ExternalInput")
    tags_d = nc.dram_tensor("tagsT", [128, NBLK], BF16, kind="ExternalInput")
    trans_d = nc.dram_tensor("transT", [K, K], F32, kind="ExternalInput")
    out_d = nc.dram_tensor("out", [1, BPC], F32, kind="ExternalOutput")
    emit_d = nc.dram_tensor("emitcols", [128, NBLK], F32, kind="ExternalOutput")

    with tile.TileContext(nc) as tc:
        with (
            tc.tile_pool(name="const", bufs=1) as cpool,
            tc.tile_pool(name="big", bufs=1) as bigpool,
            tc.tile_pool(name="seg", bufs=2) as segpool,
            tc.tile_pool(name="nat", bufs=4) as natpool,
            tc.tile_pool(name="apool", bufs=3) as apool,
            tc.tile_pool(name="scr", bufs=8) as scrpool,
            tc.tile_pool(name="psum", bufs=4, space="PSUM") as psum_pool,
            tc.tile_pool(name="psumf", bufs=1, space="PSUM") as psum_fin,
        ):
            # ---- constants ----
            # transT input is transitions^T - c (host pre-shifted), so W and
            # stopcol are both exp() of it; logZ = logS + (T+1)*c on host.
            transT_s = cpool.tile([K, K], F32)
            nc.sync.dma_start(out=transT_s, in_=trans_d[:])
            W = cpool.tile([K, K], BF16)  # [prev, next] = exp(T^T - c)
            nc.scalar.activation(W, transT_s, AF.Exp)
            stopcol = cpool.tile([K, 1], F32)  # exp(T[STOP, k] - c) per partition k
            nc.scalar.activation(stopcol, transT_s[:, STOP : STOP + 1], AF.Exp)
            ones_b = cpool.tile([K, 1], BF16)
            nc.vector.memset(ones_b, 1.0)
            iota_k = cpool.tile([K, K], BF16)  # iota_k[p, j] = j
            nc.gpsimd.iota(
                iota_k,
                pattern=[[1, K]],
                base=0,
                channel_multiplier=0,
                allow_small_or_imprecise_dtypes=True,
            )
            tags_s = cpool.tile([128, NBLK], BF16)
            nc.sync.dma_start(out=tags_s, in_=tags_d[:])
            emit_cols = bigpool.tile([128, NBLK], F32)

            # ---- resident transposed feats, t-major: col = t*BPC + b ----
            # Chunked plain DMAs so segment 0 is ready within a few us.
            featsT = bigpool.tile([K, T * BPC], BF16)
            seg_cols = TSEG * BPC
            for s in range(NSEG):
                nc.sync.dma_start(
                    out=featsT[:, s * seg_cols : (s + 1) * seg_cols],
                    in_=featsT_d[:, s * seg_cols : (s + 1) * seg_cols],
                )

            # natural-layout feats blocks for the emit score (scalar engine
            # HWDGE queue so the sync queue stays on the scan-critical loads)
            nat_tiles = []
            for g in range(NBLK // BLK_GRP):
                nat = natpool.tile([128, BLK_GRP, K], BF16)
                nc.scalar.dma_start(
                    out=nat,
                    in_=featsN_d[
                        g * BLK_GRP * 128 : (g + 1) * BLK_GRP * 128, :
                    ].rearrange("(j p) k -> p j k", j=BLK_GRP),
                )
                nat_tiles.append(nat)

            # ---- A0 = onehot(START): fill 1.0 where partition == START ----
            A = apool.tile([K, BPC], BF16)
            nc.gpsimd.memset(A, 0.0)
            nc.gpsimd.affine_select(
                out=A,
                in_=A,
                compare_op=AluOpType.not_equal,
                fill=1.0,
                base=-START,
                channel_multiplier=1,
                pattern=[[0, BPC]],
            )

            # ---- the scan, with emit ops interleaved 1 per 2 steps ----
            def emit_op(col):
                g, j = divmod(col, BLK_GRP)
                scr = scrpool.tile([128, K], BF16, name="scr")
                nc.vector.scalar_tensor_tensor(
                    out=scr,
                    in0=iota_k,
                    scalar=tags_s[:, col : col + 1],
                    in1=nat_tiles[g][:, j, :],
                    op0=AluOpType.is_equal,
                    op1=AluOpType.mult,
                    accum_out=emit_cols[:, col : col + 1],
                )

            emit_idx = 0
            for s in range(NSEG):
                expF = segpool.tile([K, TSEG * BPC], F32)
                nc.scalar.activation(
                    expF, featsT[:, s * seg_cols : (s + 1) * seg_cols], AF.Exp
                )
                for ti in range(TSEG):
                    psum_M = psum_pool.tile([K, BPC], F32)
                    nc.tensor.matmul(psum_M, W, A, start=True, stop=True)
                    A_new = apool.tile([K, BPC], BF16, name="A_new")
                    nc.vector.tensor_mul(
                        A_new, psum_M, expF[:, ti * BPC : (ti + 1) * BPC]
                    )
                    A = A_new
                    t_global = s * TSEG + ti
                    if t_global % 2 == 1 and emit_idx < NBLK:
                        emit_op(emit_idx)
                        emit_idx += 1
            while emit_idx < NBLK:
                emit_op(emit_idx)
                emit_idx += 1

            # ---- finalize: logS = log(colsum(A ⊙ stopcol)) ----
            Afin = apool.tile([K, BPC], BF16)
            nc.vector.tensor_scalar_mul(Afin, A, stopcol)
            psum_S = psum_fin.tile([1, BPC], F32)
            nc.tensor.matmul(psum_S, ones_b, Afin, start=True, stop=True)
            logS = cpool.tile([1, BPC], F32)
            nc.scalar.activation(logS, psum_S, AF.Ln)
            nc.sync.dma_start(out=out_d[:], in_=logS)
            nc.sync.dma_start(out=emit_d[:], in_=emit_cols)

    nc.compile()
    nc.finalize()
    _NC_CACHE[key] = nc
    return nc


def prep_inputs(feats, tags, transitions):
    """Host-side marshalling: slice per core, cast bf16, build both layouts."""
    feats_bf = np.asarray(feats, dtype=np.float32).astype(ml_dtypes.bfloat16)
    tags64 = np.asarray(tags).astype(np.int64)
    transT = np.ascontiguousarray(
        np.asarray(transitions, dtype=np.float32).T - np.float32(C_SHIFT)
    )
    in_maps = []
    for c in range(NCORES):
        fc = feats_bf[c * BPC : (c + 1) * BPC]  # [BPC, T, K]
        fT = np.ascontiguousarray(fc.transpose(2, 1, 0).reshape(K, T * BPC))
        fN = np.ascontiguousarray(fc.reshape(BPC * T, K))
        tg = np.ascontiguousarray(
            tags64[c * BPC : (c + 1) * BPC]
            .reshape(NBLK, 128)
            .T.astype(ml_dtypes.bfloat16)
        )
        in_maps.append({"featsT": fT, "featsN": fN, "tagsT": tg, "transT": transT})
    return in_maps, tags64


def combine_outputs(results, tags64, transitions):
    """Host-side: per-core logS/emit partials + trans gold score -> loss."""
    Trf = np.asarray(transitions, dtype=np.float64)
    ext = np.concatenate([np.full((B, 1), START, np.int64), tags64], axis=1)
    trans_gold = Trf[ext[:, 1:], ext[:, :-1]].sum(axis=1) + Trf[STOP, ext[:, -1]]
    total = 0.0
    for c in range(NCORES):
        logS = results[c]["out"][0].astype(np.float64)  # [BPC]
        ecols = results[c]["emitcols"].astype(np.float64)  # [128, NBLK]
        emit_b = ecols.sum(axis=0).reshape(BPC, 4).sum(axis=1)
        logZ = logS + (T + 1) * C_SHIFT
        total += float(np.sum(logZ - emit_b - trans_gold[c * BPC : (c + 1) * BPC]))
    return np.asarray(total / B, dtype=np.float32)


def kernel(feats, tags, transitions):
    from concourse.bass_utils import run_bass_kernel_spmd

    nc = build_kernel()
    in_maps, tags64 = prep_inputs(feats, tags, transitions)
    res = run_bass_kernel_spmd(nc, in_maps, list(range(NCORES)))
    return combine_outputs(res.results, tags64, transitions)


if __name__ == "__main__":
    nc = build_kernel()
    print("kernel built and compiled OK")


# revision 13
# speedup vs baseline: 1.5102x; 1.2829x over previous
"""Trainium2 Bass kernel for nn_BiLSTM_CRF (CRF negative log-likelihood loss).

Problem: loss = mean_b( logZ_b - gold_b ) for a linear-chain CRF with
B=512 sequences, T=512 steps, K=128 tags (START=126, STOP=127).

Algorithm (per core, data-parallel over batch, 64 sequences/core):
  The log-semiring forward scan is computed in the exp domain so each step
  is one 128x128x64 TensorE matmul with a *fixed* stationary weight
  W = exp(transitions^T - c), where c is a constant per-step shift that
  keeps exp-domain magnitudes in fp32/bf16 range (the per-step log-growth
  of the partition function is ~c; measured drift stays within +-7 log
  units over all 512 steps, far inside bf16/fp32 exponent range):

      A_0 = onehot(START);  A_{t+1} = exp(feats_t) ⊙ (W @ A_t)
      logZ = log(colsum(A_T ⊙ exp(T[STOP,:] - c))) + (T+1)*c

  Gold-path score splits into:
    - emit  = sum_t feats[b,t,tags[b,t]]      -> on device (touches feats):
      one fused DVE scalar_tensor_tensor per 128-row block:
      (iota_k == tag_p) * feats_nat with accum_out giving the free-dim sum.
      Emit ops are interleaved 1:2 with scan steps so they fill the DVE
      gaps between the scan's PSUM-evacuation multiplies.
    - trans = sum_t T[tag_t,tag_{t-1}] (+STOP) -> on host (64KB table gather).

feats is shipped twice in bf16 (transposed [K, t-major(T,B)] for the scan's
matmul/exp pipeline, natural [B*T, K] for emit) -- 16MB/core of DMA, fully
hidden under the ~512-step scan chain.

The final mean over batch is a host-side sum of the 8 per-core partials.
"""

import numpy as np
import ml_dtypes

import concourse.bass as bass
from concourse import bacc
import concourse.mybir as mybir
import concourse.tile as tile
from concourse.tile import add_dep_helper
from concourse.alu_op_type import AluOpType

B, T, K = 512, 512, 128
NCORES = 8
BPC = B // NCORES  # 64 sequences per core
START, STOP = K - 2, K - 1

# Constant per-step shift: E[logZ]/T measured on the problem's data
# distribution (randn feats/transitions). Any value within ~0.1 of the true
# mean growth keeps the scan in range; measured drift with this value is
# [-6.7, +5.9] log units.
C_SHIFT = 5.826096

TSEG = 32               # scan timesteps per exp() segment
NSEG = T // TSEG
NBLK = BPC * T // 128   # 256 natural-layout 128-row blocks for emit score
BLK_GRP = 8             # natural blocks DMA'd together
F32 = mybir.dt.float32
BF16 = mybir.dt.bfloat16

_NC_CACHE = {}


def build_kernel():
    key = "nc"
    if key in _NC_CACHE:
        return _NC_CACHE[key]
    nc = bacc.Bacc(None, target_bir_lowering=False)
    AF = mybir.ActivationFunctionType

    featsT_d = nc.dram_tensor("featsT", [K, T * BPC], BF16, kind="ExternalInput")
    featsN_d = nc.dram_tensor("featsN", [BPC * T, K], BF16, kind="ExternalInput")
    tags_d = nc.dram_tensor("tagsT", [128, NBLK], BF16, kind="ExternalInput")
    trans_d = nc.dram_tensor("transT", [K, K], F32, kind="ExternalInput")
    out_d = nc.dram_tensor("out", [1, BPC], F32, kind="ExternalOutput")
    emit_d = nc.dram_tensor("emitcols", [128, NBLK], F32, kind="ExternalOutput")

    with tile.TileContext(nc) as tc:
        with (
            tc.tile_pool(name="const", bufs=1) as cpool,
            tc.tile_pool(name="big", bufs=1) as bigpool,
            tc.tile_pool(name="seg", bufs=2) as segpool,
            tc.tile_pool(name="nat", bufs=4) as natpool,
            tc.tile_pool(name="apool", bufs=3) as apool,
            tc.tile_pool(name="scr", bufs=8) as scrpool,
            tc.tile_pool(name="psum", bufs=4, space="PSUM") as psum_pool,
            tc.tile_pool(name="psumf", bufs=1, space="PSUM") as psum_fin,
        ):
            # ---- constants ----
            # transT input is transitions^T - c (host pre-shifted), so W and
            # stopcol are both exp() of it; logZ = logS + (T+1)*c on host.
            transT_s = cpool.tile([K, K], F32)
            nc.sync.dma_start(out=transT_s, in_=trans_d[:])
            W = cpool.tile([K, K], BF16)  # [prev, next] = exp(T^T - c)
            nc.scalar.activation(W, transT_s, AF.Exp)
            stopcol = cpool.tile([K, 1], F32)  # exp(T[STOP, k] - c) per partition k
            nc.scalar.activation(stopcol, transT_s[:, STOP : STOP + 1], AF.Exp)
            ones_b = cpool.tile([K, 1], BF16)
            nc.vector.memset(ones_b, 1.0)
            iota_k = cpool.tile([K, K], BF16)  # iota_k[p, j] = j
            nc.gpsimd.iota(
                iota_k,
                pattern=[[1, K]],
                base=0,
                channel_multiplier=0,
                allow_small_or_imprecise_dtypes=True,
            )
            tags_s = cpool.tile([128, NBLK], BF16)
            nc.sync.dma_start(out=tags_s, in_=tags_d[:])
            emit_cols = bigpool.tile([128, NBLK], F32)

            # ---- resident transposed feats, t-major: col = t*BPC + b ----
            # Chunked plain DMAs so segment 0 is ready within a few us.
            featsT = bigpool.tile([K, T * BPC], BF16)
            seg_cols = TSEG * BPC
            for s in range(NSEG):
                nc.sync.dma_start(
                    out=featsT[:, s * seg_cols : (s + 1) * seg_cols],
                    in_=featsT_d[:, s * seg_cols : (s + 1) * seg_cols],
                )

            # natural-layout feats blocks for the emit score (scalar engine
            # HWDGE queue so the sync queue stays on the scan-critical loads)
            nat_tiles = []
            for g in range(NBLK // BLK_GRP):
                nat = natpool.tile([128, BLK_GRP, K], BF16)
                nc.scalar.dma_start(
                    out=nat,
                    in_=featsN_d[
                        g * BLK_GRP * 128 : (g + 1) * BLK_GRP * 128, :
                    ].rearrange("(j p) k -> p j k", j=BLK_GRP),
                )
                nat_tiles.append(nat)

            # ---- A0 = onehot(START): fill 1.0 where partition == START ----
            A = apool.tile([K, BPC], BF16)
            nc.gpsimd.memset(A, 0.0)
            nc.gpsimd.affine_select(
                out=A,
                in_=A,
                compare_op=AluOpType.not_equal,
                fill=1.0,
                base=-START,
                channel_multiplier=1,
                pattern=[[0, BPC]],
            )

            # ---- the scan, with emit ops interleaved 1 per 2 steps ----
            # An explicit (non-sem) scheduler dep from each emit op onto the
            # preceding scan multiply keeps the DVE queue alternating
            # scan/emit; without it the scheduler front-loads all 256 emit
            # ops, stalling the scan chain ~90us.
            def emit_op(col, after_inst):
                g, j = divmod(col, BLK_GRP)
                scr = scrpool.tile([128, K], BF16, name="scr")
                ei = nc.vector.scalar_tensor_tensor(
                    out=scr,
                    in0=iota_k,
                    scalar=tags_s[:, col : col + 1],
                    in1=nat_tiles[g][:, j, :],
                    op0=AluOpType.is_equal,
                    op1=AluOpType.mult,
                    accum_out=emit_cols[:, col : col + 1],
                )
                if after_inst is not None:
                    add_dep_helper(
                        ei.ins, after_inst.ins, sync=False,
                        reason="spread emit over scan gaps",
                    )

            emit_idx = 0
            for s in range(NSEG):
                expF = segpool.tile([K, TSEG * BPC], F32)
                nc.scalar.activation(
                    expF, featsT[:, s * seg_cols : (s + 1) * seg_cols], AF.Exp
                )
                for ti in range(TSEG):
                    psum_M = psum_pool.tile([K, BPC], F32)
                    nc.tensor.matmul(psum_M, W, A, start=True, stop=True)
                    A_new = apool.tile([K, BPC], BF16, name="A_new")
                    mi = nc.vector.tensor_mul(
                        A_new, psum_M, expF[:, ti * BPC : (ti + 1) * BPC]
                    )
                    A = A_new
                    t_global = s * TSEG + ti
                    if t_global % 2 == 1 and emit_idx < NBLK:
                        emit_op(emit_idx, mi)
                        emit_idx += 1
            while emit_idx < NBLK:
                emit_op(emit_idx, None)
                emit_idx += 1

            # ---- finalize: logS = log(colsum(A ⊙ stopcol)) ----
            Afin = apool.tile([K, BPC], BF16)
            nc.vector.tensor_scalar_mul(Afin, A, stopcol)
            psum_S = psum_fin.tile([1, BPC], F32)
            nc.tensor.matmul(psum_S, ones_b, Afin, start=True, stop=True)
            logS = cpool.tile([1, BPC], F32)
            nc.scalar.activation(logS, psum_S, AF.Ln)
            nc.sync.dma_start(out=out_d[:], in_=logS)
            nc.sync.dma_start(out=emit_d[:], in_=emit_cols)

    nc.compile()
    nc.finalize()
    _NC_CACHE[key] = nc
    return nc


def prep_inputs(feats, tags, transitions):
    """Host-side marshalling: slice per core, cast bf16, build both layouts."""
    feats_bf = np.asarray(feats, dtype=np.float32).astype(ml_dtypes.bfloat16)
    tags64 = np.asarray(tags).astype(np.int64)
    transT = np.ascontiguousarray(
        np.asarray(transitions, dtype=np.float32).T - np.float32(C_SHIFT)
    )
    in_maps = []
    for c in range(NCORES):
        fc = feats_bf[c * BPC : (c + 1) * BPC]  # [BPC, T, K]
        fT = np.ascontiguousarray(fc.transpose(2, 1, 0).reshape(K, T * BPC))
        fN = np.ascontiguousarray(fc.reshape(BPC * T, K))
        tg = np.ascontiguousarray(
            tags64[c * BPC : (c + 1) * BPC]
            .reshape(NBLK, 128)
            .T.astype(ml_dtypes.bfloat16)
        )
        in_maps.append({"featsT": fT, "featsN": fN, "tagsT": tg, "transT": transT})
    return in_maps, tags64


def combine_outputs(results, tags64, transitions):
    """Host-side: per-core logS/emit partials + trans gold score -> loss."""
    Trf = np.asarray(transitions, dtype=np.float64)
    ext = np.concatenate([np.full((B, 1), START, np.int64), tags64], axis=1)
    trans_gold = Trf[ext[:, 1:], ext[:, :-1]].sum(axis=1) + Trf[STOP, ext[:, -1]]
    total = 0.0
    for c in range(NCORES):
        logS = results[c]["out"][0].astype(np.float64)  # [BPC]
        ecols = results[c]["emitcols"].astype(np.float64)  # [128, NBLK]
        emit_b = ecols.sum(axis=0).reshape(BPC, 4).sum(axis=1)
        logZ = logS + (T + 1) * C_SHIFT
        total += float(np.sum(logZ - emit_b - trans_gold[c * BPC : (c + 1) * BPC]))
    return np.asarray(total / B, dtype=np.float32)


def kernel(feats, tags, transitions):
    from concourse.bass_utils import run_bass_kernel_spmd

    nc = build_kernel()
    in_maps, tags64 = prep_inputs(feats, tags, transitions)
    res = run_bass_kernel_spmd(nc, in_maps, list(range(NCORES)))
    return combine_outputs(res.results, tags64, transitions)


if __name__ == "__main__":
    nc = build_kernel()
    print("kernel built and compiled OK")


# revision 15
# speedup vs baseline: 1.5228x; 1.0083x over previous
"""Trainium2 Bass kernel for nn_BiLSTM_CRF (CRF negative log-likelihood loss).

Problem: loss = mean_b( logZ_b - gold_b ) for a linear-chain CRF with
B=512 sequences, T=512 steps, K=128 tags (START=126, STOP=127).

Algorithm (per core, data-parallel over batch, 64 sequences/core):
  The log-semiring forward scan is computed in the exp domain so each step
  is one 128x128x64 TensorE matmul with a *fixed* stationary weight
  W = exp(transitions^T - c), where c is a constant per-step shift that
  keeps exp-domain magnitudes in fp32/bf16 range (the per-step log-growth
  of the partition function is ~c; measured drift stays within +-7 log
  units over all 512 steps, far inside bf16/fp32 exponent range):

      A_0 = onehot(START);  A_{t+1} = exp(feats_t) ⊙ (W @ A_t)
      logZ = log(colsum(A_T ⊙ exp(T[STOP,:] - c))) + (T+1)*c

  Gold-path score splits into:
    - emit  = sum_t feats[b,t,tags[b,t]]      -> on device (touches feats):
      one fused DVE scalar_tensor_tensor per 128-row block:
      (iota_k == tag_p) * feats_nat with accum_out giving the free-dim sum.
      Emit ops are interleaved 1:2 with scan steps so they fill the DVE
      gaps between the scan's PSUM-evacuation multiplies.
    - trans = sum_t T[tag_t,tag_{t-1}] (+STOP) -> on host (64KB table gather).

feats is shipped twice in bf16 (transposed [K, t-major(T,B)] for the scan's
matmul/exp pipeline, natural [B*T, K] for emit) -- 16MB/core of DMA, fully
hidden under the ~512-step scan chain.

The final mean over batch is a host-side sum of the 8 per-core partials.
"""

import numpy as np
import ml_dtypes

import concourse.bass as bass
from concourse import bacc
import concourse.mybir as mybir
import concourse.tile as tile
from concourse.tile import add_dep_helper
from concourse.alu_op_type import AluOpType

B, T, K = 512, 512, 128
NCORES = 8
BPC = B // NCORES  # 64 sequences per core
START, STOP = K - 2, K - 1

# Constant per-step shift: E[logZ]/T measured on the problem's data
# distribution (randn feats/transitions). Any value within ~0.1 of the true
# mean growth keeps the scan in range; measured drift with this value is
# [-6.7, +5.9] log units.
C_SHIFT = 5.826096

TSEG = 32               # scan timesteps per exp() segment
NSEG = T // TSEG
NBLK = BPC * T // 128   # 256 natural-layout 128-row blocks for emit score
BLK_GRP = 8             # natural blocks DMA'd together
F32 = mybir.dt.float32
BF16 = mybir.dt.bfloat16

_NC_CACHE = {}


def build_kernel():
    key = "nc"
    if key in _NC_CACHE:
        return _NC_CACHE[key]
    nc = bacc.Bacc(None, target_bir_lowering=False)
    AF = mybir.ActivationFunctionType

    featsT_d = nc.dram_tensor("featsT", [K, T * BPC], BF16, kind="ExternalInput")
    featsN_d = nc.dram_tensor("featsN", [BPC * T, K], BF16, kind="ExternalInput")
    tags_d = nc.dram_tensor("tagsT", [128, NBLK], BF16, kind="ExternalInput")
    trans_d = nc.dram_tensor("transT", [K, K], F32, kind="ExternalInput")
    out_d = nc.dram_tensor("out", [1, BPC], F32, kind="ExternalOutput")
    emit_d = nc.dram_tensor("emitcols", [128, NBLK], F32, kind="ExternalOutput")

    with tile.TileContext(nc) as tc:
        with (
            tc.tile_pool(name="const", bufs=1) as cpool,
            tc.tile_pool(name="big", bufs=1) as bigpool,
            tc.tile_pool(name="seg", bufs=2) as segpool,
            tc.tile_pool(name="nat", bufs=4) as natpool,
            tc.tile_pool(name="apool", bufs=3) as apool,
            tc.tile_pool(name="scr", bufs=8) as scrpool,
            tc.tile_pool(name="psum", bufs=4, space="PSUM") as psum_pool,
            tc.tile_pool(name="psumf", bufs=1, space="PSUM") as psum_fin,
        ):
            # ---- constants ----
            # transT input is transitions^T - c (host pre-shifted), so W and
            # stopcol are both exp() of it; logZ = logS + (T+1)*c on host.
            transT_s = cpool.tile([K, K], F32)
            nc.sync.dma_start(out=transT_s, in_=trans_d[:])
            W = cpool.tile([K, K], BF16)  # [prev, next] = exp(T^T - c)
            nc.scalar.activation(W, transT_s, AF.Exp)
            stopcol = cpool.tile([K, 1], F32)  # exp(T[STOP, k] - c) per partition k
            nc.scalar.activation(stopcol, transT_s[:, STOP : STOP + 1], AF.Exp)
            ones_b = cpool.tile([K, 1], BF16)
            nc.vector.memset(ones_b, 1.0)
            iota_k = cpool.tile([K, K], BF16)  # iota_k[p, j] = j
            nc.gpsimd.iota(
                iota_k,
                pattern=[[1, K]],
                base=0,
                channel_multiplier=0,
                allow_small_or_imprecise_dtypes=True,
            )
            emit_cols = bigpool.tile([128, NBLK], F32)

            # ---- resident transposed feats, t-major: col = t*BPC + b ----
            # Chunked plain DMAs so segment 0 is ready within a few us;
            # segment 0 itself lands in 4 sub-chunks so the scan can start
            # as soon as the first 8 timesteps are in.
            featsT = bigpool.tile([K, T * BPC], BF16)
            seg_cols = TSEG * BPC
            for q in range(4):
                sub = seg_cols // 4
                nc.sync.dma_start(
                    out=featsT[:, q * sub : (q + 1) * sub],
                    in_=featsT_d[:, q * sub : (q + 1) * sub],
                )
            tags_s = cpool.tile([128, NBLK], BF16)
            nc.sync.dma_start(out=tags_s, in_=tags_d[:])
            for s in range(1, NSEG):
                nc.sync.dma_start(
                    out=featsT[:, s * seg_cols : (s + 1) * seg_cols],
                    in_=featsT_d[:, s * seg_cols : (s + 1) * seg_cols],
                )

            # natural-layout feats blocks for the emit score (scalar engine
            # HWDGE queue so the sync queue stays on the scan-critical loads)
            nat_tiles = []
            for g in range(NBLK // BLK_GRP):
                nat = natpool.tile([128, BLK_GRP, K], BF16)
                nc.scalar.dma_start(
                    out=nat,
                    in_=featsN_d[
                        g * BLK_GRP * 128 : (g + 1) * BLK_GRP * 128, :
                    ].rearrange("(j p) k -> p j k", j=BLK_GRP),
                )
                nat_tiles.append(nat)

            # ---- A0 = onehot(START): fill 1.0 where partition == START ----
            A = apool.tile([K, BPC], BF16)
            nc.gpsimd.memset(A, 0.0)
            nc.gpsimd.affine_select(
                out=A,
                in_=A,
                compare_op=AluOpType.not_equal,
                fill=1.0,
                base=-START,
                channel_multiplier=1,
                pattern=[[0, BPC]],
            )

            # ---- the scan, with emit ops interleaved 1 per 2 steps ----
            # An explicit (non-sem) scheduler dep from each emit op onto the
            # preceding scan multiply keeps the DVE queue alternating
            # scan/emit; without it the scheduler front-loads all 256 emit
            # ops, stalling the scan chain ~90us.
            def emit_op(col, after_inst):
                g, j = divmod(col, BLK_GRP)
                scr = scrpool.tile([128, K], BF16, name="scr")
                ei = nc.vector.scalar_tensor_tensor(
                    out=scr,
                    in0=iota_k,
                    scalar=tags_s[:, col : col + 1],
                    in1=nat_tiles[g][:, j, :],
                    op0=AluOpType.is_equal,
                    op1=AluOpType.mult,
                    accum_out=emit_cols[:, col : col + 1],
                )
                if after_inst is not None:
                    add_dep_helper(
                        ei.ins, after_inst.ins, sync=False,
                        reason="spread emit over scan gaps",
                    )

            emit_idx = 0
            for s in range(NSEG):
                expF = segpool.tile([K, TSEG * BPC], F32)
                if s == 0:
                    for q in range(4):
                        sub = seg_cols // 4
                        nc.scalar.activation(
                            expF[:, q * sub : (q + 1) * sub],
                            featsT[:, q * sub : (q + 1) * sub],
                            AF.Exp,
                        )
                else:
                    nc.scalar.activation(
                        expF, featsT[:, s * seg_cols : (s + 1) * seg_cols], AF.Exp
                    )
                for ti in range(TSEG):
                    psum_M = psum_pool.tile([K, BPC], F32)
                    nc.tensor.matmul(psum_M, W, A, start=True, stop=True)
                    A_new = apool.tile([K, BPC], BF16, name="A_new")
                    mi = nc.vector.tensor_mul(
                        A_new, psum_M, expF[:, ti * BPC : (ti + 1) * BPC]
                    )
                    A = A_new
                    t_global = s * TSEG + ti
                    if t_global % 2 == 1 and emit_idx < NBLK:
                        emit_op(emit_idx, mi)
                        emit_idx += 1
            while emit_idx < NBLK:
                emit_op(emit_idx, None)
                emit_idx += 1

            # ---- finalize: logS = log(colsum(A ⊙ stopcol)) ----
            Afin = apool.tile([K, BPC], BF16)
            nc.vector.tensor_scalar_mul(Afin, A, stopcol)
            psum_S = psum_fin.tile([1, BPC], F32)
            nc.tensor.matmul(psum_S, ones_b, Afin, start=True, stop=True)
            logS = cpool.tile([1, BPC], F32)
            nc.scalar.activation(logS, psum_S, AF.Ln)
            nc.sync.dma_start(out=out_d[:], in_=logS)
            nc.sync.dma_start(out=emit_d[:], in_=emit_cols)

    nc.compile()
    nc.finalize()
    _NC_CACHE[key] = nc
    return nc


def prep_inputs(feats, tags, transitions):
    """Host-side marshalling: slice per core, cast bf16, build both layouts."""
    feats_bf = np.asarray(feats, dtype=np.float32).astype(ml_dtypes.bfloat16)
    tags64 = np.asarray(tags).astype(np.int64)
    transT = np.ascontiguousarray(
        np.asarray(transitions, dtype=np.float32).T - np.float32(C_SHIFT)
    )
    in_maps = []
    for c in range(NCORES):
        fc = feats_bf[c * BPC : (c + 1) * BPC]  # [BPC, T, K]
        fT = np.ascontiguousarray(fc.transpose(2, 1, 0).reshape(K, T * BPC))
        fN = np.ascontiguousarray(fc.reshape(BPC * T, K))
        tg = np.ascontiguousarray(
            tags64[c * BPC : (c + 1) * BPC]
            .reshape(NBLK, 128)
            .T.astype(ml_dtypes.bfloat16)
        )
        in_maps.append({"featsT": fT, "featsN": fN, "tagsT": tg, "transT": transT})
    return in_maps, tags64


def combine_outputs(results, tags64, transitions):
    """Host-side: per-core logS/emit partials + trans gold score -> loss."""
    Trf = np.asarray(transitions, dtype=np.float64)
    ext = np.concatenate([np.full((B, 1), START, np.int64), tags64], axis=1)
    trans_gold = Trf[ext[:, 1:], ext[:, :-1]].sum(axis=1) + Trf[STOP, ext[:, -1]]
    total = 0.0
    for c in range(NCORES):
        logS = results[c]["out"][0].astype(np.float64)  # [BPC]
        ecols = results[c]["emitcols"].astype(np.float64)  # [128, NBLK]
        emit_b = ecols.sum(axis=0).reshape(BPC, 4).sum(axis=1)
        logZ = logS + (T + 1) * C_SHIFT
        total += float(np.sum(logZ - emit_b - trans_gold[c * BPC : (c + 1) * BPC]))
    return np.asarray(total / B, dtype=np.float32)


def kernel(feats, tags, transitions):
    from concourse.bass_utils import run_bass_kernel_spmd

    nc = build_kernel()
    in_maps, tags64 = prep_inputs(feats, tags, transitions)
    res = run_bass_kernel_spmd(nc, in_maps, list(range(NCORES)))
    return combine_outputs(res.results, tags64, transitions)


if __name__ == "__main__":
    nc = build_kernel()
    print("kernel built and compiled OK")


# revision 19
# speedup vs baseline: 1.6556x; 1.0872x over previous
"""Trainium2 Bass kernel for nn_BiLSTM_CRF (CRF negative log-likelihood loss).

Problem: loss = mean_b( logZ_b - gold_b ) for a linear-chain CRF with
B=512 sequences, T=512 steps, K=128 tags (START=126, STOP=127).

Algorithm (per core, data-parallel over batch, 64 sequences/core):
  The log-semiring forward scan is computed in the exp domain so each step
  is one 128x128x64 TensorE matmul with a *fixed* stationary weight
  W = exp(transitions^T - c), where c is a constant per-step shift that
  keeps exp-domain magnitudes in fp32/bf16 range (the per-step log-growth
  of the partition function is ~c; measured drift stays within +-7 log
  units over all 512 steps, far inside bf16/fp32 exponent range):

      A_0 = onehot(START);  A_{t+1} = exp(feats_t) ⊙ (W @ A_t)
      logZ = log(colsum(A_T ⊙ exp(T[STOP,:] - c))) + (T+1)*c

  Gold-path score splits into:
    - emit  = sum_t feats[b,t,tags[b,t]]      -> on device (touches feats):
      one fused DVE scalar_tensor_tensor per 128-row block:
      (iota_k == tag_p) * feats_nat with accum_out giving the free-dim sum.
      Emit ops are interleaved 1:2 with scan steps so they fill the DVE
      gaps between the scan's PSUM-evacuation multiplies.
    - trans = sum_t T[tag_t,tag_{t-1}] (+STOP) -> on host (64KB table gather).

feats is shipped twice in bf16 (transposed [K, t-major(T,B)] for the scan's
matmul/exp pipeline, natural [B*T, K] for emit) -- 16MB/core of DMA, fully
hidden under the ~512-step scan chain.

The final mean over batch is a host-side sum of the 8 per-core partials.
"""

import numpy as np
import ml_dtypes

import concourse.bass as bass
from concourse import bacc
import concourse.mybir as mybir
import concourse.tile as tile
from concourse.tile import add_dep_helper
from concourse.alu_op_type import AluOpType

B, T, K = 512, 512, 128
NCORES = 8
BPC = B // NCORES  # 64 sequences per core
START, STOP = K - 2, K - 1

# Constant per-step shift: E[logZ]/T measured on the problem's data
# distribution (randn feats/transitions). Any value within ~0.1 of the true
# mean growth keeps the scan in range; measured drift with this value is
# [-6.7, +5.9] log units.
C_SHIFT = 5.826096

TSEG = 32               # scan timesteps per exp() segment
NSEG = T // TSEG
NBLK = BPC * T // 128   # 256 natural-layout 128-row blocks for emit score
BLK_GRP = 8             # natural blocks DMA'd together
F32 = mybir.dt.float32
BF16 = mybir.dt.bfloat16

_NC_CACHE = {}


def build_kernel():
    key = "nc"
    if key in _NC_CACHE:
        return _NC_CACHE[key]
    nc = bacc.Bacc(None, target_bir_lowering=False)
    AF = mybir.ActivationFunctionType

    featsT_d = nc.dram_tensor("featsT", [K, T * BPC], BF16, kind="ExternalInput")
    featsN_d = nc.dram_tensor("featsN", [BPC * T, K], BF16, kind="ExternalInput")
    tags_d = nc.dram_tensor("tagsT", [128, NBLK], BF16, kind="ExternalInput")
    trans_d = nc.dram_tensor("transT", [K, K], F32, kind="ExternalInput")
    out_d = nc.dram_tensor("out", [1, BPC], F32, kind="ExternalOutput")
    emit_d = nc.dram_tensor("emitcols", [128, NBLK], F32, kind="ExternalOutput")

    with tile.TileContext(nc) as tc:
        with (
            tc.tile_pool(name="const", bufs=1) as cpool,
            tc.tile_pool(name="big", bufs=1) as bigpool,
            tc.tile_pool(name="seg", bufs=2) as segpool,
            tc.tile_pool(name="nat", bufs=4) as natpool,
            tc.tile_pool(name="apool", bufs=3) as apool,
            tc.tile_pool(name="scr", bufs=8) as scrpool,
            tc.tile_pool(name="psum", bufs=3, space="PSUM") as psum_pool,
            tc.tile_pool(name="psumf", bufs=1, space="PSUM") as psum_fin,
        ):
            # ---- constants ----
            # transT input is transitions^T - c (host pre-shifted), so W and
            # stopcol are both exp() of it; logZ = logS + (T+1)*c on host.
            transT_s = cpool.tile([K, K], F32)
            nc.sync.dma_start(out=transT_s, in_=trans_d[:])
            W = cpool.tile([K, K], BF16)  # [prev, next] = exp(T^T - c)
            nc.scalar.activation(W, transT_s, AF.Exp)
            stopcol = cpool.tile([K, 1], F32)  # exp(T[STOP, k] - c) per partition k
            nc.scalar.activation(stopcol, transT_s[:, STOP : STOP + 1], AF.Exp)
            ones_b = cpool.tile([K, 1], BF16)
            nc.vector.memset(ones_b, 1.0)
            iota_k = cpool.tile([K, K], BF16)  # iota_k[p, j] = j
            nc.gpsimd.iota(
                iota_k,
                pattern=[[1, K]],
                base=0,
                channel_multiplier=0,
                allow_small_or_imprecise_dtypes=True,
            )
            emit_cols = bigpool.tile([128, NBLK], F32)

            # ---- resident transposed feats, t-major: col = t*BPC + b ----
            # Chunked plain DMAs so segment 0 is ready within a few us;
            # segment 0 itself lands in 4 sub-chunks so the scan can start
            # as soon as the first 8 timesteps are in.
            featsT = bigpool.tile([K, T * BPC], BF16)
            seg_cols = TSEG * BPC
            for q in range(4):
                sub = seg_cols // 4
                nc.sync.dma_start(
                    out=featsT[:, q * sub : (q + 1) * sub],
                    in_=featsT_d[:, q * sub : (q + 1) * sub],
                )
            tags_s = cpool.tile([128, NBLK], BF16)
            nc.sync.dma_start(out=tags_s, in_=tags_d[:])
            for s in range(1, NSEG):
                nc.sync.dma_start(
                    out=featsT[:, s * seg_cols : (s + 1) * seg_cols],
                    in_=featsT_d[:, s * seg_cols : (s + 1) * seg_cols],
                )

            # natural-layout feats blocks for the emit score (scalar engine
            # HWDGE queue so the sync queue stays on the scan-critical loads)
            nat_tiles = []
            for g in range(NBLK // BLK_GRP):
                nat = natpool.tile([128, BLK_GRP, K], BF16)
                nc.scalar.dma_start(
                    out=nat,
                    in_=featsN_d[
                        g * BLK_GRP * 128 : (g + 1) * BLK_GRP * 128, :
                    ].rearrange("(j p) k -> p j k", j=BLK_GRP),
                )
                nat_tiles.append(nat)

            # ---- A0 = onehot(START): fill 1.0 where partition == START ----
            # Two half-batch chains (32 seqs each) interleave so one chain's
            # DVE multiply overlaps the other's matmul latency.
            HB = BPC // 2
            A_half = []
            for h in range(2):
                Ah = apool.tile([K, HB], BF16, name=f"A0_{h}", tag=f"a0_{h}")
                nc.gpsimd.memset(Ah, 0.0)
                nc.gpsimd.affine_select(
                    out=Ah,
                    in_=Ah,
                    compare_op=AluOpType.not_equal,
                    fill=1.0,
                    base=-START,
                    channel_multiplier=1,
                    pattern=[[0, HB]],
                )
                A_half.append(Ah)

            # ---- the scan, with emit ops interleaved 1 per 2 steps ----
            # An explicit (non-sem) scheduler dep from each emit op onto the
            # preceding scan multiply keeps the DVE queue alternating
            # scan/emit; without it the scheduler front-loads all 256 emit
            # ops, stalling the scan chain ~90us.
            def emit_op(col, after_inst):
                g, j = divmod(col, BLK_GRP)
                scr = scrpool.tile([128, K], BF16, name="scr")
                ei = nc.vector.scalar_tensor_tensor(
                    out=scr,
                    in0=iota_k,
                    scalar=tags_s[:, col : col + 1],
                    in1=nat_tiles[g][:, j, :],
                    op0=AluOpType.is_equal,
                    op1=AluOpType.mult,
                    accum_out=emit_cols[:, col : col + 1],
                )
                if after_inst is not None:
                    add_dep_helper(
                        ei.ins, after_inst.ins, sync=False,
                        reason="spread emit over scan gaps",
                    )

            emit_idx = 0
            for s in range(NSEG):
                expF = segpool.tile([K, TSEG * BPC], F32)
                if s == 0:
                    for q in range(4):
                        sub = seg_cols // 4
                        nc.scalar.activation(
                            expF[:, q * sub : (q + 1) * sub],
                            featsT[:, q * sub : (q + 1) * sub],
                            AF.Exp,
                        )
                else:
                    nc.scalar.activation(
                        expF, featsT[:, s * seg_cols : (s + 1) * seg_cols], AF.Exp
                    )
                for ti in range(TSEG):
                    mi = None
                    for h in range(2):
                        psum_M = psum_pool.tile([K, HB], F32, name=f"pm{h}")
                        nc.tensor.matmul(
                            psum_M, W, A_half[h], start=True, stop=True
                        )
                        A_new = apool.tile(
                            [K, HB], BF16, name=f"A_new{h}", tag=f"a{h}"
                        )
                        mi = nc.vector.tensor_mul(
                            A_new,
                            psum_M,
                            expF[:, ti * BPC + h * HB : ti * BPC + (h + 1) * HB],
                        )
                        A_half[h] = A_new
                    t_global = s * TSEG + ti
                    if t_global % 2 == 1 and emit_idx < NBLK:
                        emit_op(emit_idx, mi)
                        emit_idx += 1
            while emit_idx < NBLK:
                emit_op(emit_idx, None)
                emit_idx += 1

            # ---- finalize: logS = log(colsum(A ⊙ stopcol)) ----
            Afin = apool.tile([K, BPC], BF16)
            for h in range(2):
                nc.vector.tensor_scalar_mul(
                    Afin[:, h * HB : (h + 1) * HB], A_half[h], stopcol
                )
            psum_S = psum_fin.tile([1, BPC], F32)
            nc.tensor.matmul(psum_S, ones_b, Afin, start=True, stop=True)
            logS = cpool.tile([1, BPC], F32)
            nc.scalar.activation(logS, psum_S, AF.Ln)
            nc.sync.dma_start(out=out_d[:], in_=logS)
            nc.sync.dma_start(out=emit_d[:], in_=emit_cols)

    nc.compile()
    nc.finalize()
    _NC_CACHE[key] = nc
    return nc


def prep_inputs(feats, tags, transitions):
    """Host-side marshalling: slice per core, cast bf16, build both layouts."""
    feats_bf = np.asarray(feats, dtype=np.float32).astype(ml_dtypes.bfloat16)
    tags64 = np.asarray(tags).astype(np.int64)
    transT = np.ascontiguousarray(
        np.asarray(transitions, dtype=np.float32).T - np.float32(C_SHIFT)
    )
    in_maps = []
    for c in range(NCORES):
        fc = feats_bf[c * BPC : (c + 1) * BPC]  # [BPC, T, K]
        fT = np.ascontiguousarray(fc.transpose(2, 1, 0).reshape(K, T * BPC))
        fN = np.ascontiguousarray(fc.reshape(BPC * T, K))
        tg = np.ascontiguousarray(
            tags64[c * BPC : (c + 1) * BPC]
            .reshape(NBLK, 128)
            .T.astype(ml_dtypes.bfloat16)
        )
        in_maps.append({"featsT": fT, "featsN": fN, "tagsT": tg, "transT": transT})
    return in_maps, tags64


def combine_outputs(results, tags64, transitions):
    """Host-side: per-core logS/emit partials + trans gold score -> loss."""
    Trf = np.asarray(transitions, dtype=np.float64)
    ext = np.concatenate([np.full((B, 1), START, np.int64), tags64], axis=1)
    trans_gold = Trf[ext[:, 1:], ext[:, :-1]].sum(axis=1) + Trf[STOP, ext[:, -1]]
    total = 0.0
    for c in range(NCORES):
        logS = results[c]["out"][0].astype(np.float64)  # [BPC]
        ecols = results[c]["emitcols"].astype(np.float64)  # [128, NBLK]
        emit_b = ecols.sum(axis=0).reshape(BPC, 4).sum(axis=1)
        logZ = logS + (T + 1) * C_SHIFT
        total += float(np.sum(logZ - emit_b - trans_gold[c * BPC : (c + 1) * BPC]))
    return np.asarray(total / B, dtype=np.float32)


def kernel(feats, tags, transitions):
    from concourse.bass_utils import run_bass_kernel_spmd

    nc = build_kernel()
    in_maps, tags64 = prep_inputs(feats, tags, transitions)
    res = run_bass_kernel_spmd(nc, in_maps, list(range(NCORES)))
    return combine_outputs(res.results, tags64, transitions)


if __name__ == "__main__":
    nc = build_kernel()
    print("kernel built and compiled OK")
